# revision 8
# baseline (speedup 1.0000x reference)
"""Trainium2 Bass kernel: single-layer LSTM (PyTorch gate order i,f,g,o) +
output projection, data-parallel over the batch dim across 8 NeuronCores.

Problem shapes: x [256, 1024, 64], W_ih [512, 64], W_hh [512, 128],
b_ih/b_hh [512], W_out [64, 128] -> out [256, 1024, 64], fp32.

Per-core layout ("feat-major"): every on-chip tensor of the recurrence is
[feature-partition, batch-free].  Per-core batch Bc=32.

  - xT   [65, Bc*T]  bf16: x transposed (row 64 = ones for the bias row),
         column = b*T + t.
  - WihT [65, 512]   bf16: [W_ih.T ; b_ih+b_hh], g-gate block and its bias
         pre-scaled by 2 (tanh(x) = 2*sigmoid(2x) - 1, exact in bf16).
  - WhhT [128, 512]  bf16: W_hh.T, g-gate block pre-scaled by 2.
  - gates PSUM window tile [128, 512] = (i,f,g,o) x TW timesteps x 32b.
    xg matmuls (K=65, start on first) open the accumulation group; the
    per-step recurrent matmuls (K=128, start=False) accumulate on top.
  - sc tiles (x2, parity-rotated): [128, 192] = [i|f|gs|o|g2|c]:
    ONE sigmoid writes cols 0:128 (all four gates); DVE fixup writes
    g2 = 2*gs-1 at 128:160; c state at 160:192 (written by step t into the
    tile of parity t+1), so P = sc[:,0:64] * sc[:,128:192] computes
    [i*g~ | f*c] in ONE tensor_tensor op.
  - hT [128, 32*(T+1)] bf16: hidden states (col block t+1 = h after step t).
  - projection: per TW-step window one matmul hT-block.T @ W_out.T -> PSUM
    [128(4t*32b), 64], DVE-copy to SBUF, DMA to y[b, t, f].
"""

import os
import sys
import numpy as np

if "/opt/trn_rl_repo" not in sys.path:
    sys.path.insert(0, "/opt/trn_rl_repo")

import ml_dtypes

from concourse import bass, bacc, mybir
from concourse.tile import TileContext
from concourse.bass_utils import run_bass_kernel_spmd

BF16 = ml_dtypes.bfloat16

N_CORES = 8
B, T, F, H = 256, 1024, 64, 128
BC = B // N_CORES  # 32 batch rows per core
G4 = 4 * H  # 512
TW = 4  # timesteps per PSUM gates window (one bank)

FP32 = mybir.dt.float32
BF = mybir.dt.bfloat16
AF = mybir.ActivationFunctionType
OP = mybir.AluOpType


def build_lstm_nc(t_steps=T):
    """Builds the per-core Bass program (SPMD: same program on all cores)."""
    nc = bacc.Bacc("TRN2", target_bir_lowering=False, debug=False)
    f32 = FP32

    xT_d = nc.dram_tensor("xT", (65, BC * t_steps), BF, kind="ExternalInput").ap()
    wih_d = nc.dram_tensor("WihT", (65, G4), BF, kind="ExternalInput").ap()
    whh_d = nc.dram_tensor("WhhT", (H, G4), BF, kind="ExternalInput").ap()
    wout_d = nc.dram_tensor("WoutT", (H, F), BF, kind="ExternalInput").ap()
    y_d = nc.dram_tensor("y", (BC, t_steps, F), f32, kind="ExternalOutput").ap()

    nw = t_steps // TW

    with TileContext(nc) as tc:
        with (
            tc.tile_pool(name="const", bufs=1) as cpool,
            tc.tile_pool(name="work", bufs=2) as wpool,
            tc.tile_pool(name="ystage", bufs=4) as ypool,
            tc.tile_pool(name="gates", bufs=3, space="PSUM") as gpool,
            tc.tile_pool(name="proj", bufs=2, space="PSUM") as ppool,
        ):
            # ---- resident tensors ----
            xT = cpool.tile([65, BC * t_steps], BF)
            wih = cpool.tile([65, G4], BF)
            whh = cpool.tile([H, G4], BF)
            wout = cpool.tile([H, F], BF)
            hT = cpool.tile([H, BC * (t_steps + 1)], BF)
            scA = cpool.tile([H, 192], f32)  # [i|f|gs|o|g2|c]
            scB = cpool.tile([H, 192], f32)
            sc = [scA, scB]

            nc.sync.dma_start(xT, xT_d)
            nc.sync.dma_start(wih, wih_d)
            nc.sync.dma_start(whh, whh_d)
            nc.sync.dma_start(wout, wout_d)
            nc.vector.memset(hT[:, 0:BC], 0.0)  # h_{-1} = 0
            nc.vector.memset(scA[:, 160:192], 0.0)  # c_{-1} = 0 (parity 0)

            # view of xT with free dims (t, b): col = b*t_steps + t
            xT_tb = xT.rearrange("k (b t) -> k t b", b=BC)

            for w in range(nw):
                t0 = w * TW
                # one PSUM bank; col = g*(TW*32) + tl*32 + b (gate-major)
                gates = gpool.tile([H, TW * 128], f32)
                gates_v = gates.rearrange("p (g t x) -> p g t x", g=4, t=TW)

                # xg: 4 gate matmuls covering TW steps each
                for g in range(4):
                    nc.tensor.matmul(
                        gates[:, g * TW * 32 : (g + 1) * TW * 32],
                        lhsT=wih[:, g * H : (g + 1) * H],
                        rhs=xT_tb[:, t0 : t0 + TW, :],
                        start=(g == 0),
                        stop=False,
                        skip_group_check=True,
                    )

                for tl in range(TW):
                    t = t0 + tl
                    s0 = sc[t % 2]  # this step's gates + [g2|c_in]
                    s1 = sc[(t + 1) % 2]  # c_out goes here
                    # recurrent matmuls accumulate on top of xg
                    for g in range(4):
                        nc.tensor.matmul(
                            gates[
                                :,
                                g * TW * 32 + tl * 32 : g * TW * 32 + (tl + 1) * 32,
                            ],
                            lhsT=whh[:, g * H : (g + 1) * H],
                            rhs=hT[:, t * BC : (t + 1) * BC],
                            start=False,
                            stop=(tl == TW - 1 and g == 3),
                            skip_group_check=True,
                        )
                    # ONE sigmoid over all four gates (g-gate pre-scaled by 2)
                    nc.scalar.activation(
                        s0[:, 0:128].rearrange("p (g x) -> p g x", g=4),
                        gates_v[:, :, tl, :],
                        AF.Sigmoid,
                    )
                    # g2 = 2*gs - 1  (= tanh of the pre-activation)
                    nc.vector.tensor_scalar(
                        s0[:, 128:160], s0[:, 64:96], 2.0, -1.0, OP.mult, OP.add
                    )
                    # P = [i*g2 | f*c]
                    P = wpool.tile([H, 64], f32, tag="P")
                    nc.vector.tensor_tensor(P, s0[:, 0:64], s0[:, 128:192], OP.mult)
                    # c' = i*g2 + f*c  -> written into the OTHER parity tile
                    nc.vector.tensor_tensor(
                        s1[:, 160:192], P[:, 0:32], P[:, 32:64], OP.add
                    )
                    # th = tanh(c')
                    th = wpool.tile([H, 32], f32, tag="th")
                    nc.scalar.activation(th, s1[:, 160:192], AF.Tanh)
                    # h = o * th  (bf16, written into hT col block t+1)
                    nc.vector.tensor_tensor(
                        hT[:, (t + 1) * BC : (t + 2) * BC],
                        s0[:, 96:128],
                        th,
                        OP.mult,
                    )

                # projection of this window's hidden states
                proj = ppool.tile([TW * BC, F], f32)
                nc.tensor.matmul(
                    proj,
                    lhsT=hT[:, (t0 + 1) * BC : (t0 + 1 + TW) * BC],
                    rhs=wout,
                    start=True,
                    stop=True,
                )
                yst = ypool.tile([TW * BC, F], f32, tag="yst")
                nc.vector.tensor_copy(yst, proj)
                # DMA out: partition p = tl*BC + b -> y[b, t0+tl, :]
                nc.sync.dma_start(
                    y_d[:, t0 : t0 + TW, :].rearrange("b t f -> t b f"), yst
                )

    nc.compile()
    return nc


def host_prep(x, W_ih, W_hh, b_ih, b_hh, W_out, t_steps=T):
    """Shard + preprocess inputs into per-core in_maps (layout only + dtype
    casts; all model arithmetic stays on device)."""
    bias = (b_ih + b_hh).astype(np.float32)  # [512]
    wihT = np.concatenate(
        [W_ih.T.astype(np.float32), bias[None, :]], axis=0
    )  # [65, 512]
    whhT = W_hh.T.astype(np.float32)  # [128, 512]
    # pre-scale the g-gate block (cols 256:384 = ref rows 256:384)
    wihT[:, 2 * H : 3 * H] *= 2.0
    whhT[:, 2 * H : 3 * H] *= 2.0
    woutT = W_out.T.astype(np.float32)  # [128, 64]

    wihT = wihT.astype(BF16)
    whhT = whhT.astype(BF16)
    woutT = woutT.astype(BF16)

    in_maps = []
    for c in range(N_CORES):
        xs = x[c * BC : (c + 1) * BC, :t_steps, :]  # [BC, t, F]
        xT = np.empty((65, BC * t_steps), dtype=BF16)
        xT[:64] = xs.transpose(2, 0, 1).reshape(64, BC * t_steps).astype(BF16)
        xT[64] = BF16(1.0)
        in_maps.append({"xT": xT, "WihT": wihT, "WhhT": whhT, "WoutT": woutT})
    return in_maps


_CACHED = {}


def kernel(x, W_ih, W_hh, b_ih, b_hh, W_out):
    x = np.asarray(x, dtype=np.float32)
    in_maps = host_prep(
        x,
        np.asarray(W_ih, np.float32),
        np.asarray(W_hh, np.float32),
        np.asarray(b_ih, np.float32),
        np.asarray(b_hh, np.float32),
        np.asarray(W_out, np.float32),
    )
    if "nc" not in _CACHED:
        _CACHED["nc"] = build_lstm_nc()
    nc = _CACHED["nc"]
    res = run_bass_kernel_spmd(nc, in_maps, core_ids=list(range(N_CORES)))
    out = np.concatenate([r["y"] for r in res.results], axis=0)
    return out.astype(np.float32)


if __name__ == "__main__":
    nc = build_lstm_nc()
    print("built ok")
